# revision 2
# baseline (speedup 1.0000x reference)
"""HeteroClassifier GNN kernel for 8 TRN2 NeuronCores (Bass/Tile), v2.

Wire-lean design: each edge costs ~5 bytes on the host->device tunnel
(2B gather idx + 2B bf16 coef + 1B target lane), vs ~65B in v1.

Sharding: L1 edges by dst node-range (owner core aggregates its nodes);
L2 edges by src node-range (g-table gathers stay core-local); per-core
[B,2] rank-layout partials are unpermuted and summed on the host.

Mechanism per stream: gpsimd ap_gather from an SBUF-replicated bf16
table using group-shared int16 index streams; a dense masked coef
(built on-device from packed [8,S] coef + lane streams via one
DRAM->DRAM broadcast DMA and an is_equal mask pass) routes each edge's
value to its target lane; a single uniform-length windowed reduce per
stream lands sums in natural (lane, window) order.

Host folds all normalization products (ew*ods*ids for L1, ids/cnt for
L2), W2_r@Wc, and the bias terms, so the device computes only
gather/mask/reduce plus the tiny per-node dense math.
"""

import numpy as np
import ml_dtypes

import concourse.bass as bass
import concourse.bacc as bacc
import concourse.mybir as mybir
import concourse.tile as tile
from concourse import bass_utils

LAST_EXEC_NS = -1
LAST_TRACE = None

N = 200000
R = 4
E = 1000000
B = 1024
NCORES = 8
P = 128
NB = N // NCORES          # 25000 nodes per core
NW1 = (NB + P - 1) // P   # 196 windows
NSLOT = NW1 * P           # 25088 node slots per core
CH = 25000                # gather-table chunk rows (N/CH = 8 chunks)
NCH = N // CH
NW2 = B // P              # 8 graph windows
MSL = 24576               # mask-pass slice columns

BF16 = ml_dtypes.bfloat16


def _round4(x):
    return ((int(x) + 3) // 4) * 4


def _round8(x):
    # L2 segment lengths must be mult-of-8 so the half-gather idx slice
    # starts on an even int16 column (the gather idx AP needs 4B align)
    return ((int(x) + 7) // 8) * 8


def _prep(feat, src, dst, ew, graph_ids):
    """Build per-core packed streams. Returns (per_core, meta, host)."""
    src = np.asarray(src)
    dst = np.asarray(dst)
    ew = np.asarray(ew, dtype=np.float32)
    gid = np.asarray(graph_ids)
    feat = np.asarray(feat, dtype=np.float32)

    ods = np.empty((R, N), dtype=np.float32)
    ids = np.empty((R, N), dtype=np.float32)
    for r in range(R):
        od = np.bincount(src[r], minlength=N)
        idg = np.bincount(dst[r], minlength=N)
        ods[r] = 1.0 / np.sqrt(np.clip(od, 1, None))
        ids[r] = 1.0 / np.sqrt(np.clip(idg, 1, None))
    cnt = np.bincount(gid, minlength=B)
    inv_cnt = 1.0 / np.clip(cnt, 1, None).astype(np.float32)

    # ---- L1: per-chunk bucket counts -> shared uniform L per chunk
    core_l = [dst[r] // NB for r in range(R)]
    dl_l = [dst[r] - core_l[r] * NB for r in range(R)]
    ch_l = [src[r] // CH for r in range(R)]
    # bucket key: (core, ch, g, win)
    NBUK = NCORES * NCH * 8 * NW1
    Lch = np.zeros(NCH, dtype=np.int64)
    keys = []
    for r in range(R):
        lane = dl_l[r] % P
        win = dl_l[r] // P
        g = lane // 16
        key = ((core_l[r] * NCH + ch_l[r]) * 8 + g) * NW1 + win
        keys.append(key)
        bc = np.bincount(key, minlength=NBUK).reshape(NCORES, NCH, 8 * NW1)
        Lch = np.maximum(Lch, bc.max(axis=(0, 2)))
    Lch = np.array([_round4(v) for v in Lch], dtype=np.int64)
    Sch = 4 * NW1 * Lch                       # stream cols per group per chunk
    offL1 = np.concatenate([[0], np.cumsum(Sch)])

    # ---- L2: shared rank map over graphs
    tot = np.zeros(B, dtype=np.int64)
    for r in range(R):
        tot += np.bincount(gid[dst[r]], minlength=B)
    order = np.argsort(-tot, kind="stable")
    rank = np.empty(B, dtype=np.int64)
    rank[order] = np.arange(B)
    # rank k -> (lane, win)
    bk = rank % 64
    g2_of = bk % 8
    w2_of = bk // 8
    li_of = rank // 64
    lane2_of = g2_of * 16 + li_of
    row2_of = lane2_of * NW2 + w2_of          # out_part row per graph

    L2 = np.zeros(R, dtype=np.int64)
    core2_l = [src[r] // NB for r in range(R)]
    for r in range(R):
        kk = rank[gid[dst[r]]]
        buk = (core2_l[r] * 64) + (kk % 64)
        bc = np.bincount(buk, minlength=NCORES * 64)
        L2[r] = max(L2[r], bc.max())
    L2 = np.array([_round8(v) for v in L2], dtype=np.int64)
    S2 = NW2 * L2
    offL2 = offL1[-1] + np.concatenate([[0], np.cumsum(S2)])
    STOT = int(offL2[-1])
    STOTP = ((STOT + MSL - 1) // MSL) * MSL   # pad so mask slices are uniform

    ncol = STOTP // 16
    idx_t = [np.zeros((P, ncol), dtype=np.int16) for _ in range(NCORES)]
    cfp = [np.zeros((8, STOTP), dtype=BF16) for _ in range(NCORES)]
    lnp = [np.zeros((8, STOTP), dtype=np.uint8) for _ in range(NCORES)]

    def scatter(core, g, j, rowv, cfv, lnv, base):
        jj = base + j
        for c in range(NCORES):
            m = core == c
            gj, jm = g[m], jj[m]
            idx_t[c][gj * 16 + jm % 16, jm // 16] = rowv[m]
            cfp[c][gj, jm] = cfv[m]
            lnp[c][gj, jm] = lnv[m]

    # ---- L1 streams
    for r in range(R):
        lane = dl_l[r] % P
        win = dl_l[r] // P
        g = lane // 16
        l16 = (lane % 16).astype(np.uint8)
        row = (src[r] - ch_l[r] * CH).astype(np.int16)
        cf = (ew[r] * ods[r][src[r]] * ids[r][dst[r]]).astype(BF16)
        # position within bucket, bucket = (core, ch, g, win)
        key = keys[r]
        sidx = np.argsort(key, kind="stable")
        ks = key[sidx]
        starts = np.searchsorted(ks, np.arange(NBUK))
        pos = np.empty(len(ks), dtype=np.int64)
        pos[sidx] = np.arange(len(ks)) - starts[ks]
        L_e = Lch[ch_l[r]]
        j = (win * 4 + r) * L_e + pos
        base = offL1[ch_l[r]]
        scatter(core_l[r], g, j, row, cf, l16, base)

    # ---- L2 streams
    for r in range(R):
        d = dst[r]
        kk = rank[gid[d]]
        g = (kk % 64) % 8
        w = (kk % 64) // 8
        li = (kk // 64).astype(np.uint8)
        n = src[r] - core2_l[r] * NB
        row = ((n % P) * NW1 + n // P).astype(np.int16)
        cf = (ids[r][d] * inv_cnt[gid[d]]).astype(BF16)
        buk = (core2_l[r] * 64) + (kk % 64)
        sidx = np.argsort(buk, kind="stable")
        ks = buk[sidx]
        starts = np.searchsorted(ks, np.arange(NCORES * 64))
        pos = np.empty(len(ks), dtype=np.int64)
        pos[sidx] = np.arange(len(ks)) - starts[ks]
        j = w * L2[r] + pos
        scatter(core2_l[r], g, j, row, cf, li, int(offL2[r]))

    meta = {
        "Lch": Lch.tolist(), "Sch": Sch.tolist(), "offL1": offL1.tolist(),
        "L2": L2.tolist(), "S2": S2.tolist(), "offL2": offL2.tolist(),
        "STOTP": int(STOTP),
    }
    host = {
        "row2_of": row2_of, "cnt": cnt,
        "ods": ods, "feat": feat,
    }
    per_core = [{"idxall": idx_t[c], "cfp": cfp[c], "lnp": lnp[c]}
                for c in range(NCORES)]
    return per_core, meta, host


def _build_program(meta):
    nc = bacc.Bacc("TRN2", target_bir_lowering=False, debug=False,
                   num_devices=NCORES)
    f32, i16, u8, bf16 = (mybir.dt.float32, mybir.dt.int16, mybir.dt.uint8,
                          mybir.dt.bfloat16)
    AL = mybir.AluOpType
    Lch, Sch, offL1 = meta["Lch"], meta["Sch"], meta["offL1"]
    L2, S2, offL2 = meta["L2"], meta["S2"], meta["offL2"]
    STOTP = meta["STOTP"]
    ncol = STOTP // 16

    featB = nc.dram_tensor("featB", [N * 2], bf16, kind="ExternalInput").ap()
    idxall = nc.dram_tensor("idxall", [P, ncol], i16, kind="ExternalInput").ap()
    cfpD = nc.dram_tensor("cfp", [8, STOTP], bf16, kind="ExternalInput").ap()
    lnpD = nc.dram_tensor("lnp", [8, STOTP], u8, kind="ExternalInput").ap()
    lncD = nc.dram_tensor("lnc", [P, 1], bf16, kind="ExternalInput").ap()
    w1pD = nc.dram_tensor("w1p", [P, 128], f32, kind="ExternalInput").ap()
    b1sD = nc.dram_tensor("b1s", [P, 16], f32, kind="ExternalInput").ap()
    mallD = nc.dram_tensor("mall", [P, 128], f32, kind="ExternalInput").ap()
    odslD = nc.dram_tensor("odsl", [P, R * NW1], f32, kind="ExternalInput").ap()

    lndD = nc.dram_tensor("lnd", [P, STOTP], u8, kind="Internal").ap()
    cfdD = nc.dram_tensor("cfd", [P, STOTP], bf16, kind="Internal").ap()
    cfmD = nc.dram_tensor("cfm", [P, STOTP], bf16, kind="Internal").ap()
    gtabD = [nc.dram_tensor(f"gtab{r}", [NSLOT * 2], bf16, kind="Internal").ap()
             for r in range(R)]
    outD = nc.dram_tensor("out_part", [B, 2], f32, kind="ExternalOutput").ap()
    import os as _os
    dbg = bool(_os.environ.get("K_DEBUG"))
    if dbg:
        dbgX = nc.dram_tensor("dbg_x", [P, 4 * NW1 * 2], mybir.dt.float32,
                              kind="ExternalOutput").ap()
        dbgH = nc.dram_tensor("dbg_h1", [P, NW1 * 16], mybir.dt.float32,
                              kind="ExternalOutput").ap()
        dbgG = nc.dram_tensor("dbg_g", [P, R * NW1 * 2], mybir.dt.float32,
                              kind="ExternalOutput").ap()
        dbgP = nc.dram_tensor("dbg_pr", [P, R * NW2 * 2], mybir.dt.float32,
                              kind="ExternalOutput").ap()
        dbgC = nc.dram_tensor("dbg_cfm0", [P, MSL], mybir.dt.bfloat16,
                              kind="ExternalOutput").ap()

    with tile.TileContext(nc) as tc:
        with tc.tile_pool(name="glob", bufs=1) as gp:
            lnc = gp.tile([P, 1], bf16, name="lnc")
            nc.sync.dma_start(out=lnc[:], in_=lncD[:])

            # ---- phase 0: expand cf/ln to dense DRAM; build masked coef
            # (sliced: a single whole-tensor expand trips the >3-dim DMA
            # AP balancer)
            with tc.tile_pool(name="mp", bufs=1) as mp:
                nsl = STOTP // MSL
                for s in range(nsl):
                    sl = slice(s * MSL, (s + 1) * MSL)
                    nc.sync.dma_start(
                        out=lndD[:, sl].rearrange("(g x) s -> g x s", x=16),
                        in_=lnpD[:, None, sl].to_broadcast([8, 16, MSL]))
                    nc.sync.dma_start(
                        out=cfdD[:, sl].rearrange("(g x) s -> g x s", x=16),
                        in_=cfpD[:, None, sl].to_broadcast([8, 16, MSL]))
                    cfb = mp.tile([P, MSL], bf16, name=f"cfb{s}", tag="cfb")
                    nc.sync.dma_start(out=cfb[:], in_=cfdD[:, sl])
                    lnb = mp.tile([P, MSL], u8, name=f"lnb{s}", tag="lnb")
                    nc.sync.dma_start(out=lnb[:], in_=lndD[:, sl])
                    eqm = mp.tile([P, MSL], bf16, name=f"eqm{s}", tag="eqm")
                    nc.vector.tensor_tensor(
                        out=eqm[:], in0=lnb[:],
                        in1=lnc[:, 0:1].to_broadcast([P, MSL]), op=AL.is_equal)
                    nc.vector.tensor_tensor(out=eqm[:], in0=eqm[:],
                                            in1=cfb[:], op=AL.mult)
                    nc.sync.dma_start(out=cfmD[:, sl], in_=eqm[:])
                    if dbg and s == 0:
                        nc.sync.dma_start(out=dbgC[:, :], in_=cfmD[:, sl])

            # ---- phase 1: L1 gather chunks -> x [P, 784, 2]
            x_t = gp.tile([P, 4 * NW1, 2], f32, name="x_t")
            with tc.tile_pool(name="p1", bufs=1) as p1:
                for ch in range(NCH):
                    S = Sch[ch]
                    tab = p1.tile([P, CH, 2], bf16, name=f"tab{ch}", tag="tab")
                    nc.sync.dma_start(
                        out=tab[:].rearrange("p n c -> p (n c)"),
                        in_=featB[ch * 2 * CH:(ch + 1) * 2 * CH][None, :]
                        .to_broadcast([P, 2 * CH]))
                    c0 = offL1[ch] // 16
                    idx = p1.tile([P, S // 16], i16, name=f"ix{ch}", tag="ix")
                    nc.sync.dma_start(out=idx[:],
                                      in_=idxall[:, c0:c0 + S // 16])
                    for h in range(2):
                        Sh = S // 2
                        cfm = p1.tile([P, Sh], bf16, name=f"cf{ch}_{h}",
                                      tag="cf")
                        nc.sync.dma_start(
                            out=cfm[:],
                            in_=cfmD[:, offL1[ch] + h * Sh:
                                     offL1[ch] + (h + 1) * Sh])
                        go = p1.tile([P, Sh, 2], bf16, name=f"go{ch}_{h}",
                                     tag="go")
                        nc.gpsimd.ap_gather(
                            out_ap=go[:, :, :], in_ap=tab[:, :, :],
                            idxs_ap=idx[:, h * (Sh // 16):(h + 1) * (Sh // 16)],
                            channels=P, num_elems=CH, d=2, num_idxs=Sh)
                        nc.vector.tensor_tensor(
                            out=go[:, :, :], in0=go[:, :, :],
                            in1=cfm[:, :, None].to_broadcast([P, Sh, 2]),
                            op=AL.mult)
                        qsl = slice(h * 2 * NW1, (h + 1) * 2 * NW1)
                        if ch == 0:
                            nc.vector.tensor_reduce(
                                out=x_t[:, qsl, :],
                                in_=go[:, :, :].rearrange(
                                    "p (q l) c -> p q c l", l=Lch[ch]),
                                op=AL.add, axis=mybir.AxisListType.X)
                        else:
                            tmp = p1.tile([P, 2 * NW1, 2], f32,
                                          name=f"tm{ch}_{h}", tag="tm")
                            nc.vector.tensor_reduce(
                                out=tmp[:, :, :],
                                in_=go[:, :, :].rearrange(
                                    "p (q l) c -> p q c l", l=Lch[ch]),
                                op=AL.add, axis=mybir.AxisListType.X)
                            nc.vector.tensor_add(out=x_t[:, qsl, :],
                                                 in0=x_t[:, qsl, :],
                                                 in1=tmp[:, :, :])

            if dbg:
                nc.sync.dma_start(
                    out=dbgX[:, :], in_=x_t[:].rearrange("p q c -> p (q c)"))

            # ---- phase 2: h1 = relu(x@W1 + b1s); g_r = (h1@m_r)*ods
            with tc.tile_pool(name="p2", bufs=1) as p2:
                w1 = p2.tile([P, 128], f32, name="w1")
                nc.sync.dma_start(out=w1[:], in_=w1pD[:, :])
                b1 = p2.tile([P, 16], f32, name="b1")
                nc.sync.dma_start(out=b1[:], in_=b1sD[:, :])
                mall = p2.tile([P, 128], f32, name="mall")
                nc.sync.dma_start(out=mall[:], in_=mallD[:, :])
                odsl = p2.tile([P, R * NW1], f32, name="odsl")
                nc.sync.dma_start(out=odsl[:], in_=odslD[:, :])
                h1 = p2.tile([P, NW1, 16], f32, name="h1")
                tmp8 = p2.tile([P, NW1, 8], f32, name="tmp8")
                x8 = x_t[:].rearrange("p (n r) c -> p n (r c)", r=4)
                for f in range(16):
                    nc.vector.tensor_tensor(
                        out=tmp8[:, :, :], in0=x8,
                        in1=w1[:, f * 8:(f + 1) * 8][:, None, :]
                        .to_broadcast([P, NW1, 8]), op=AL.mult)
                    nc.vector.tensor_reduce(
                        out=h1[:, :, f:f + 1], in_=tmp8[:, :, :],
                        op=AL.add, axis=mybir.AxisListType.X)
                nc.vector.tensor_tensor(
                    out=h1[:, :, :], in0=h1[:, :, :],
                    in1=b1[:, None, :].to_broadcast([P, NW1, 16]), op=AL.add)
                nc.vector.tensor_scalar_max(h1[:, :, :], h1[:, :, :], 0.0)
                if dbg:
                    nc.sync.dma_start(
                        out=dbgH[:, :],
                        in_=h1[:].rearrange("p n f -> p (n f)"))
                tmp16 = p2.tile([P, NW1, 16], f32, name="tmp16")
                for r in range(R):
                    g_t = p2.tile([P, NW1, 2], f32, name=f"g{r}", tag="g")
                    for c2 in range(2):
                        nc.vector.tensor_tensor(
                            out=tmp16[:, :, :], in0=h1[:, :, :],
                            in1=mall[:, (r * 2 + c2) * 16:
                                     (r * 2 + c2 + 1) * 16][:, None, :]
                            .to_broadcast([P, NW1, 16]), op=AL.mult)
                        nc.vector.tensor_reduce(
                            out=g_t[:, :, c2:c2 + 1], in_=tmp16[:, :, :],
                            op=AL.add, axis=mybir.AxisListType.X)
                    g_b = p2.tile([P, NW1, 2], bf16, name=f"gb{r}", tag="gb")
                    nc.vector.tensor_tensor(
                        out=g_b[:, :, :], in0=g_t[:, :, :],
                        in1=odsl[:, r * NW1:(r + 1) * NW1, None]
                        .to_broadcast([P, NW1, 2]), op=AL.mult)
                    if dbg:
                        dgt = p2.tile([P, NW1, 2], f32, name=f"dgt{r}",
                                      tag="dgt")
                        nc.vector.tensor_copy(out=dgt[:], in_=g_b[:])
                        nc.sync.dma_start(
                            out=dbgG[:, r * NW1 * 2:(r + 1) * NW1 * 2],
                            in_=dgt[:].rearrange("p n c -> p (n c)"))
                    nc.sync.dma_start(
                        out=gtabD[r].rearrange("(p k c) -> p (k c)", p=P,
                                               k=NW1),
                        in_=g_b[:].rearrange("p k c -> p (k c)"))

            # ---- phase 3: L2 gathers -> osum [P, NW2, 2]
            osum = gp.tile([P, NW2, 2], f32, name="osum")
            with tc.tile_pool(name="p3", bufs=1) as p3:
                for r in range(R):
                    S = S2[r]
                    tab2 = p3.tile([P, NSLOT, 2], bf16, name=f"t2{r}",
                                   tag="t2")
                    nc.sync.dma_start(
                        out=tab2[:].rearrange("p n c -> p (n c)"),
                        in_=gtabD[r][None, :].to_broadcast([P, NSLOT * 2]))
                    c0 = offL2[r] // 16
                    idx = p3.tile([P, S // 16], i16, name=f"ix2{r}", tag="ix2")
                    nc.sync.dma_start(out=idx[:],
                                      in_=idxall[:, c0:c0 + S // 16])
                    cfm = p3.tile([P, S], bf16, name=f"cf2{r}", tag="cf2")
                    nc.sync.dma_start(out=cfm[:],
                                      in_=cfmD[:, offL2[r]:offL2[r] + S])
                    pr = p3.tile([P, NW2, 2], f32, name=f"pr{r}", tag="pr")
                    for h in range(2):
                        Sh = S // 2
                        go2 = p3.tile([P, Sh, 2], bf16, name=f"go2{r}_{h}",
                                      tag="go2")
                        nc.gpsimd.ap_gather(
                            out_ap=go2[:, :, :], in_ap=tab2[:, :, :],
                            idxs_ap=idx[:, h * (Sh // 16):(h + 1) * (Sh // 16)],
                            channels=P, num_elems=NSLOT, d=2, num_idxs=Sh)
                        nc.vector.tensor_tensor(
                            out=go2[:, :, :], in0=go2[:, :, :],
                            in1=cfm[:, h * Sh:(h + 1) * Sh, None]
                            .to_broadcast([P, Sh, 2]), op=AL.mult)
                        nc.vector.tensor_reduce(
                            out=pr[:, h * (NW2 // 2):(h + 1) * (NW2 // 2), :],
                            in_=go2[:, :, :].rearrange(
                                "p (q l) c -> p q c l", l=L2[r]),
                            op=AL.add, axis=mybir.AxisListType.X)
                    if dbg:
                        nc.sync.dma_start(
                            out=dbgP[:, r * NW2 * 2:(r + 1) * NW2 * 2],
                            in_=pr[:].rearrange("p k c -> p (k c)"))
                    if r == 0:
                        nc.vector.tensor_copy(out=osum[:, :, :],
                                              in_=pr[:, :, :])
                    else:
                        nc.vector.tensor_add(out=osum[:, :, :],
                                             in0=osum[:, :, :],
                                             in1=pr[:, :, :])
            nc.sync.dma_start(
                out=outD.rearrange("(p k) c -> p k c", p=P),
                in_=osum[:, :, :])
    nc.compile()
    return nc


def kernel(feat, src, dst, ew, graph_ids, W1, b1, W2, b2, Wc, bc):
    per_core, meta, host = _prep(feat, src, dst, ew, graph_ids)
    nc = _build_program(meta)

    featB = np.ascontiguousarray(host["feat"], dtype=np.float32) \
        .astype(BF16).reshape(-1)
    lncv = (np.arange(P) % 16).astype(BF16)[:, None]
    W1f = np.asarray(W1, dtype=np.float32)
    w1p = np.tile(W1f.transpose(2, 0, 1).reshape(1, -1), (P, 1)) \
        .astype(np.float32)
    b1s = np.tile(np.asarray(b1, np.float32).sum(axis=0)[None, :], (P, 1))
    m_all = np.einsum("rij,jc->ric", np.asarray(W2, np.float32),
                      np.asarray(Wc, np.float32))      # [R,16,2]
    mall = np.tile(m_all.transpose(0, 2, 1).reshape(1, -1), (P, 1)) \
        .astype(np.float32)                            # (r, c2, f)
    ods = host["ods"]
    odsl = np.zeros((P, R * NW1), dtype=np.float32)

    in_maps = []
    for c in range(NCORES):
        od_c = np.zeros((R, NSLOT), dtype=np.float32)
        od_c[:, :NB] = ods[:, c * NB:(c + 1) * NB]
        # node n -> (lane n%P, win n//P): odsl[p, r*NW1 + k] = od_c[r, k*P+p]
        oc = od_c.reshape(R, NW1, P).transpose(2, 0, 1).reshape(P, R * NW1)
        in_maps.append({
            "featB": featB, "idxall": per_core[c]["idxall"],
            "cfp": per_core[c]["cfp"], "lnp": per_core[c]["lnp"],
            "lnc": lncv, "w1p": w1p, "b1s": b1s, "mall": mall,
            "odsl": np.ascontiguousarray(oc),
        })

    import os as _os
    import time as _t
    _t0 = _t.perf_counter()
    res = bass_utils.run_bass_kernel_spmd(
        nc, in_maps, core_ids=list(range(NCORES)),
        tmpdir=_os.environ.get("K_TRACE_DIR") or None)
    global LAST_EXEC_NS, LAST_TRACE
    LAST_EXEC_NS = int((_t.perf_counter() - _t0) * 1e9)
    if res.exec_time_ns:
        LAST_EXEC_NS = int(res.exec_time_ns)
    LAST_TRACE = res.instructions_and_trace[1] if res.instructions_and_trace else None

    row2 = host["row2_of"]
    acc = np.zeros((B, 2), dtype=np.float32)
    for c in range(NCORES):
        acc += res.results[c]["out_part"]
    out = acc[row2]
    # bias: non-empty graphs get (sum_r b2)@Wc + bc; empty get bc
    b2s = np.asarray(b2, np.float32).sum(axis=0)
    bias_full = b2s @ np.asarray(Wc, np.float32) + np.asarray(bc, np.float32)
    out = out + np.where((host["cnt"] > 0)[:, None], bias_full[None, :],
                         np.asarray(bc, np.float32)[None, :])
    return out.astype(np.float32)
